# revision 15
# baseline (speedup 1.0000x reference)
"""Trainium2 Bass kernel for nn_Block_31954556682442 (spiking MoE-SSA block).

Sharding: pure data-parallel over batch B=8 -> one sample (4 LIF time steps)
per NeuronCore, zero collectives. v2 design:
  - all weight matmuls as bf16 hi/lo split (3-term W@x for fc1/kq/v with
    bf16-split activations; 2-term for proj/fc2 whose rhs are exact bf16
    integers), residual error ~2^-18 -> no spike flips observed
  - time steps batched into matmul free dims (N=512 covers 2 steps)
  - bf16 exact-integer attention core (spikes are {0,1})
  - LIF scans in 2^t-scaled form: membrane update = tensor_add on GPSIMD,
    spike/reset = tensor_scalar/scalar_tensor_tensor on DVE (threshold 2^t)
  - depthwise 3x3 conv t-batched: 9 shifted per-partition-scalar MACs over
    (128, 4*256) tiles on DVE, 2^t applied at the LIF add
  - PSUM evicts fused with BN scale+bias (+2^t*0.5) on ScalarE
Self-contained: hardcodes all shapes; no sibling imports.
"""
import numpy as np
import ml_dtypes

import concourse.bacc as bacc
import concourse.mybir as mybir
import concourse.tile as tile
from concourse.bass_utils import run_bass_kernel_spmd

F32 = mybir.dt.float32
BF16 = mybir.dt.bfloat16
AL = mybir.AluOpType
AF = mybir.ActivationFunctionType

T, B, C, N = 4, 8, 384, 256
ED = 96
NE = 4
NU = 5
HID, HH = 2048, 1024
S = float(1.0 / np.sqrt(1.0 + 1e-5))
P = 128


def _body(nc, tc, d):
    from contextlib import ExitStack
    VE = nc.vector
    GE = nc.gpsimd

    with ExitStack() as ctx:
        def pool(name, bufs, space="SBUF"):
            return ctx.enter_context(tc.tile_pool(name=name, bufs=bufs, space=space))

        wp = pool("wp", 1)
        mp = pool("mp", 1)
        ps_m = pool("ps_m", 2, "PSUM")
        ps_o = pool("ps_o", 6, "PSUM")
        xs_p = pool("xs_p", 3)       # (128,1024) f32, doubles as x_new
        sphl_p = pool("sphl_p", 3)   # bf16 hi splits
        splo_p = pool("splo_p", 3)   # bf16 lo splits
        xkq_p = pool("xkq_p", 2)     # (96,1280) f32
        xev_p = pool("xev_p", 4)     # (128,768) f32 evict/LIF targets
        xrt_p = pool("xrt_p", 2)     # (128,8)
        sp_p = pool("sp_p", 4)       # (96,1280) bf16 kq spikes
        vsp_p = pool("vsp_p", 4)     # (128,768) bf16
        wsp_p = pool("wsp_p", 4)     # (128,8) f32
        at_p = pool("at_p", 3)       # (128,256) bf16
        rsp_p = pool("rsp_p", 2)     # (128,768) bf16
        y_p = pool("y_p", 8)         # (128,384) bf16
        ydn_p = pool("ydn_p", 3)     # (128,1024) bf16
        xh_p = pool("xh_p", 2)       # (128,2048) f32
        spch_p = pool("spch_p", 2)   # (128,2048) bf16
        acc_p = pool("acc_p", 2)     # (128,1024) f32
        mg_p = pool("mg_p", 2)       # (128,1024) bf16
        mh_p = pool("mh_p", 2)       # (128,512) f32
        mdw_p = pool("mdw_p", 2)     # (128,256) f32

        # ---------------- weight loads ----------------
        def wload(name, shape, dt=F32, src=None):
            w = wp.tile(shape, dt, name=name, tag=name)
            nc.sync.dma_start(out=w, in_=d[name] if src is None else src)
            return w

        ident = wload('ident', [P, P], BF16)
        # PE warmup: ~60 dummy matmuls to flip HAM to K=8/8 before phase A
        pwarm = ps_m.tile([P, P], F32, name="pwarm", tag="pm")
        for wi in range(60):
            nc.tensor.matmul(pwarm, ident, ident, start=True, stop=True)
        warm_sink = wp.tile([P, 1], F32, name="warm_sink", tag="warm_sink")
        nc.scalar.activation(warm_sink, pwarm[:, 0:1], AF.Copy)

        # xs first (A-phase starts on these)
        xs_kt = []
        for kt in range(3):
            x_ = xs_p.tile([P, 4 * N], F32, name=f"xs{kt}", tag="t")
            xs_kt.append(x_)
        for t in range(T):
            for kt in range(3):
                nc.sync.dma_start(out=xs_kt[kt][:, t*N:(t+1)*N],
                                  in_=d['xin'][t*C + kt*P: t*C + (kt+1)*P, :])
        kqh, kql, vh, vl, r_w = [], [], [], [], []
        for kt in range(3):
            kqh.append(wload(f'kqh{kt}', [P, 480], BF16, d['kq_whi'][kt*P:(kt+1)*P, :]))
            kql.append(wload(f'kql{kt}', [P, 480], BF16, d['kq_wlo'][kt*P:(kt+1)*P, :]))
        a_kq = wload('a_kq', [96, 20]); b_kq = wload('b_kq', [96, 20])
        for kt in range(3):
            vh.append(wload(f'vh{kt}', [P, 384], BF16, d['v_whi'][kt*P:(kt+1)*P, :]))
            vl.append(wload(f'vl{kt}', [P, 384], BF16, d['v_wlo'][kt*P:(kt+1)*P, :]))
            r_w.append(wload(f'r_w{kt}', [P, 4], F32, d['r_wT'][kt*P:(kt+1)*P, :]))
        rb = wload('r_b', [1, 4]); ones = wload('ones', [1, P])
        pjh, pjl, f1h, f1l, f2h, f2l = [], [], [], [], [], []
        for kt in range(3):
            pjh.append(wload(f'pjh{kt}', [P, 384], BF16, d['pj_whi'][kt*P:(kt+1)*P, :]))
            pjl.append(wload(f'pjl{kt}', [P, 384], BF16, d['pj_wlo'][kt*P:(kt+1)*P, :]))
        a_p = wload('a_p', [P, 12]); b_p = wload('b_p', [P, 12])
        for kt in range(3):
            f1h.append(wload(f'f1h{kt}', [P, 2048], BF16, d['f1_whi'][kt*P:(kt+1)*P, :]))
            f1l.append(wload(f'f1l{kt}', [P, 2048], BF16, d['f1_wlo'][kt*P:(kt+1)*P, :]))
        a_h = wload('a_h', [P, 64]); b_h = wload('b_h', [P, 64])
        dwt = wload('dw_tap', [P, 72]); dwtn = wload('dw_tapn', [P, 72])
        b_dw = wload('b_dw', [P, 8])
        for ch in range(8):
            f2h.append(wload(f'f2h{ch}', [P, 384], BF16, d['f2_whi'][ch*P:(ch+1)*P, :]))
            f2l.append(wload(f'f2l{ch}', [P, 384], BF16, d['f2_wlo'][ch*P:(ch+1)*P, :]))
        a_o = wload('a_o', [P, 12]); b_o = wload('b_o', [P, 12])

        # ---------------- xs bf16 splits ----------------
        xhi, xlo = [], []
        for kt in range(3):
            h_ = sphl_p.tile([P, 4 * N], BF16, name=f"xhi{kt}", tag="t")
            nc.scalar.activation(h_, xs_kt[kt], AF.Copy)
            l_ = splo_p.tile([P, 4 * N], BF16, name=f"xlo{kt}", tag="t")
            GE.tensor_sub(l_, xs_kt[kt], h_)
            xhi.append(h_); xlo.append(l_)

        # ---------------- phase A: kq / v / router matmuls + evicts ----------------
        m_kq = mp.tile([96, 5 * N], F32, name="m_kq", tag="m_kq")
        m_vt = mp.tile([P, 768], F32, name="m_vt", tag="m_vt")
        m_rt = mp.tile([P, 8], F32, name="m_rt", tag="m_rt")
        m_p = mp.tile([P, 768], F32, name="m_p", tag="m_p")
        m_o = mp.tile([P, 768], F32, name="m_o", tag="m_o")

        xkq_t = [xkq_p.tile([96, 5 * N], F32, name=f"xkq{t}", tag="t") for t in range(T)]
        xvt_t = [xev_p.tile([P, 768], F32, name=f"xvt{t}", tag="t") for t in range(T)]
        xrt_t = [xrt_p.tile([P, 8], F32, name=f"xrt{t}", tag="t") for t in range(T)]

        for tp in range(2):
            for u in range(NU):
                pt = ps_m.tile([96, 512], F32, name=f"pkq{u}_{tp}", tag="pm")
                first = True
                for kt in range(3):
                    rh = xhi[kt][:, tp*512:(tp+1)*512]
                    rl = xlo[kt][:, tp*512:(tp+1)*512]
                    for w_, r_ in ((kqh[kt], rh), (kqh[kt], rl), (kql[kt], rh)):
                        nc.tensor.matmul(pt, w_[:, 96*u:96*(u+1)], r_,
                                         start=first, stop=(kt == 2 and r_ is rh and w_ is kql[kt]))
                        first = False
                for ti in range(2):
                    t = tp * 2 + ti
                    c = u * 4 + t
                    nc.scalar.activation(xkq_t[t][:, u*N:(u+1)*N], pt[:, ti*N:(ti+1)*N],
                                         AF.Identity, bias=b_kq[:, c:c+1], scale=a_kq[:, c:c+1])
        for t in range(T):
            for mt in range(2):
                pv = ps_m.tile([P, 384], F32, name=f"pvt{t}_{mt}", tag="pm")
                first = True
                for kt in range(3):
                    lh = xhi[kt][:, t*N + mt*P: t*N + (mt+1)*P]
                    ll = xlo[kt][:, t*N + mt*P: t*N + (mt+1)*P]
                    for l_, w_ in ((lh, vh[kt]), (ll, vh[kt]), (lh, vl[kt])):
                        nc.tensor.matmul(pv, l_, w_, start=first,
                                         stop=(kt == 2 and l_ is lh and w_ is vl[kt]))
                        first = False
                nc.scalar.activation(xvt_t[t][:, mt*384:(mt+1)*384], pv, AF.Copy,
                                     bias=0.0, scale=0.5 * float(2.0 ** t))
            for mt in range(2):
                pr = ps_m.tile([P, 4], F32, name=f"prt{t}_{mt}", tag="pm")
                for kt in range(3):
                    nc.tensor.matmul(pr, xs_kt[kt][:, t*N + mt*P: t*N + (mt+1)*P],
                                     r_w[kt], start=(kt == 0), stop=False)
                nc.tensor.matmul(pr, ones, rb, start=False, stop=True)
                nc.scalar.activation(xrt_t[t][:, mt*4:(mt+1)*4], pr, AF.Copy,
                                     bias=0.0, scale=float(2.0 ** t))

        # ---------------- phase B: LIF scans for kq / v / r ----------------
        sp_t, v_sp, w_sp = [], [], []
        for t in range(T):
            thr = float(2.0 ** t)
            U = xkq_t[t]
            if t > 0:
                GE.tensor_add(U, m_kq, U)
            sp = sp_p.tile([96, 5 * N], BF16, name=f"sp{t}", tag="t")
            VE.tensor_single_scalar(sp, U, thr, AL.is_ge)
            if t < T - 1:
                VE.scalar_tensor_tensor(out=m_kq, in0=U, scalar=thr, in1=U,
                                        op0=AL.is_lt, op1=AL.mult)
            sp_t.append(sp)

            U = xvt_t[t]
            if t > 0:
                GE.tensor_add(U, m_vt, U)
            vs = vsp_p.tile([P, 768], BF16, name=f"vsp{t}", tag="t")
            VE.tensor_single_scalar(vs, U, thr, AL.is_ge)
            if t < T - 1:
                VE.scalar_tensor_tensor(out=m_vt, in0=U, scalar=thr, in1=U,
                                        op0=AL.is_lt, op1=AL.mult)
            v_sp.append(vs)

            U = xrt_t[t]
            if t > 0:
                GE.tensor_add(U, m_rt, U)
            ws = wsp_p.tile([P, 8], F32, name=f"wsp{t}", tag="t")
            VE.tensor_single_scalar(ws, U, thr, AL.is_ge)
            if t < T - 1:
                VE.scalar_tensor_tensor(out=m_rt, in0=U, scalar=thr, in1=U,
                                        op0=AL.is_lt, op1=AL.mult)
            w_sp.append(ws)

        # ---------------- phase C: experts ----------------
        y = [[None] * 2 for _ in range(T)]
        m_res_e = [mp.tile([P, 768], F32, name=f"m_res{e}", tag=f"m_res{e}")
                   for e in range(NE)]
        for e in range(NE):
            m_res = m_res_e[e]
            xres_e = []
            for t in range(T):
                at_sb = []
                for mt in range(2):
                    pa = ps_m.tile([P, N], F32, name=f"pat{e}{t}{mt}", tag="pm")
                    nc.tensor.matmul(pa, sp_t[t][:, mt*P:(mt+1)*P],
                                     sp_t[t][:, (1+e)*N:(2+e)*N], start=True, stop=True)
                    ats = at_p.tile([P, N], BF16, name=f"at{e}{t}{mt}", tag="t")
                    nc.scalar.activation(ats, pa, AF.Copy)
                    at_sb.append(ats)
                xr = xev_p.tile([P, 768], F32, name=f"xres{e}{t}", tag="t")
                for mt in range(2):
                    pr_ = ps_m.tile([P, 384], F32, name=f"pres{e}{t}{mt}", tag="pm")
                    for mk in range(2):
                        nc.tensor.matmul(pr_, at_sb[mk][:, mt*P:(mt+1)*P],
                                         v_sp[t][:, mk*384:(mk+1)*384],
                                         start=(mk == 0), stop=(mk == 1))
                    nc.scalar.activation(xr[:, mt*384:(mt+1)*384], pr_, AF.Copy,
                                         bias=0.0, scale=0.5 * float(2.0 ** t))
                xres_e.append(xr)
            for t in range(T):
                thr = float(2.0 ** t)
                U = xres_e[t]
                if t > 0:
                    GE.tensor_add(U, m_res, U)
                rs = rsp_p.tile([P, 768], BF16, name=f"rsp{e}{t}", tag="t")
                VE.tensor_single_scalar(rs, U, thr, AL.is_ge)
                if t < T - 1:
                    VE.scalar_tensor_tensor(out=m_res, in0=U, scalar=thr, in1=U,
                                            op0=AL.is_lt, op1=AL.mult)
                for mt in range(2):
                    if e == 0:
                        yt = y_p.tile([P, 384], BF16, name=f"y{t}_{mt}", tag="t")
                        VE.scalar_tensor_tensor(
                            out=yt, in0=rs[:, mt*384:(mt+1)*384],
                            scalar=w_sp[t][:, mt*4:mt*4+1],
                            in1=rs[:, mt*384:(mt+1)*384], op0=AL.mult, op1=AL.bypass)
                        y[t][mt] = yt
                    else:
                        VE.scalar_tensor_tensor(
                            out=y[t][mt], in0=rs[:, mt*384:(mt+1)*384],
                            scalar=w_sp[t][:, mt*4+e:mt*4+e+1],
                            in1=y[t][mt], op0=AL.mult, op1=AL.add)

        # ---------------- phase D: transpose y, proj, LIF, residual ----------------
        ydn = [ydn_p.tile([P, 4 * N], BF16, name=f"ydn{dt}", tag="t") for dt in range(3)]
        for t in range(T):
            for mt in range(2):
                for dt in range(3):
                    ptr = ps_m.tile([P, P], BF16, name=f"ptr{t}{mt}{dt}", tag="pm")
                    nc.tensor.transpose(ptr, y[t][mt][:, dt*P:(dt+1)*P], ident)
                    nc.scalar.activation(ydn[dt][:, t*N + mt*P: t*N + (mt+1)*P],
                                         ptr, AF.Copy)
        xp_t = [xev_p.tile([P, 768], F32, name=f"xp{t}", tag="t") for t in range(T)]
        for mt in range(3):
            for tp in range(2):
                pp = ps_m.tile([P, 512], F32, name=f"pp{mt}_{tp}", tag="pm")
                first = True
                for kt in range(3):
                    r_ = ydn[kt][:, tp*512:(tp+1)*512]
                    nc.tensor.matmul(pp, pjh[kt][:, mt*P:(mt+1)*P], r_,
                                     start=first, stop=False)
                    first = False
                    nc.tensor.matmul(pp, pjl[kt][:, mt*P:(mt+1)*P], r_,
                                     start=False, stop=(kt == 2))
                for ti in range(2):
                    t = tp * 2 + ti
                    c = mt * 4 + t
                    nc.scalar.activation(xp_t[t][:, mt*N:(mt+1)*N], pp[:, ti*N:(ti+1)*N],
                                         AF.Identity, bias=b_p[:, c:c+1], scale=a_p[:, c:c+1])
        for t in range(T):
            thr = float(2.0 ** t)
            U = xp_t[t]
            if t > 0:
                GE.tensor_add(U, m_p, U)
            if t < T - 1:
                VE.scalar_tensor_tensor(out=m_p, in0=U, scalar=thr, in1=U,
                                        op0=AL.is_lt, op1=AL.mult)
            for mt in range(3):
                # x_new overwrites xs in place (residual add)
                VE.scalar_tensor_tensor(
                    out=xs_kt[mt][:, t*N:(t+1)*N], in0=U[:, mt*N:(mt+1)*N],
                    scalar=thr, in1=xs_kt[mt][:, t*N:(t+1)*N],
                    op0=AL.is_ge, op1=AL.add)

        # x_new bf16 splits (reuses the split pool slots)
        xnhi, xnlo = [], []
        for kt in range(3):
            h_ = sphl_p.tile([P, 4 * N], BF16, name=f"xnhi{kt}", tag="t")
            nc.scalar.activation(h_, xs_kt[kt], AF.Copy)
            l_ = splo_p.tile([P, 4 * N], BF16, name=f"xnlo{kt}", tag="t")
            GE.tensor_sub(l_, xs_kt[kt], h_)
            xnhi.append(h_); xnlo.append(l_)

        # ---------------- phase E: MLP ----------------
        po = [[ps_o.tile([P, 512], F32, name=f"po{tp}_{mt}", tag="po")
               for mt in range(3)] for tp in range(2)]
        for ch in range(8):
            xh = xh_p.tile([P, 2048], F32, name=f"xh{ch}", tag="t")
            for half in range(2):
                mth = ch + 8 * half
                for tp in range(2):
                    ph = ps_m.tile([P, 512], F32, name=f"ph{ch}{half}{tp}", tag="pm")
                    first = True
                    for kt in range(3):
                        rh = xnhi[kt][:, tp*512:(tp+1)*512]
                        rl = xnlo[kt][:, tp*512:(tp+1)*512]
                        for w_, r_ in ((f1h[kt], rh), (f1h[kt], rl), (f1l[kt], rh)):
                            nc.tensor.matmul(ph, w_[:, mth*P:(mth+1)*P], r_,
                                             start=first,
                                             stop=(kt == 2 and r_ is rh and w_ is f1l[kt]))
                            first = False
                    for ti in range(2):
                        t = tp * 2 + ti
                        c = mth * 4 + t
                        nc.scalar.activation(
                            xh[:, half*1024 + t*N: half*1024 + (t+1)*N],
                            ph[:, ti*N:(ti+1)*N], AF.Identity,
                            bias=b_h[:, c:c+1], scale=a_h[:, c:c+1])
            # h-LIF over t (both halves via 3D APs)
            m_h = mh_p.tile([P, 512], F32, name=f"m_h{ch}", tag="t")
            sp_ch = spch_p.tile([P, 2048], BF16, name=f"spch{ch}", tag="t")
            xh3 = xh.rearrange("p (h q) -> p h q", h=2)
            mh3 = m_h.rearrange("p (h q) -> p h q", h=2)
            spc3 = sp_ch.rearrange("p (h q) -> p h q", h=2)
            for t in range(T):
                thr = float(2.0 ** t)
                U3 = xh3[:, :, t*N:(t+1)*N]
                if t > 0:
                    GE.tensor_add(U3, mh3, U3)
                VE.tensor_single_scalar(spc3[:, :, t*N:(t+1)*N], U3, thr, AL.is_ge)
                if t < T - 1:
                    VE.scalar_tensor_tensor(out=mh3, in0=U3, scalar=thr, in1=U3,
                                            op0=AL.is_lt, op1=AL.mult)
            # depthwise conv, t-batched, unscaled taps
            acc = acc_p.tile([P, 1024], F32, name=f"acc{ch}", tag="t")
            VE.tensor_scalar(acc, sp_ch[:, 0:1024], dwt[:, ch*9+4:ch*9+5],
                             b_dw[:, ch:ch+1], AL.mult, AL.add)
            x1f = sp_ch[:, 0:1024]
            x1r = x1f.rearrange("p (r w) -> p r w", w=16)     # 64 rows across t
            x1t = x1f.rearrange("p (t r) -> p t r", t=4)      # 4 t-blocks of 256
            ar = acc.rearrange("p (r w) -> p r w", w=16)
            at4 = acc.rearrange("p (t r) -> p t r", t=4)
            for dy in range(3):
                for dx in range(3):
                    if (dy, dx) == (1, 1):
                        continue
                    ct = ch * 9 + 3 * dy + dx
                    sc = dwt[:, ct:ct+1]
                    if dy == 1:
                        # pure x-shift: rows uniform across all t
                        wo0, wo1 = (1, 16) if dx == 0 else (0, 15)
                        VE.scalar_tensor_tensor(
                            out=ar[:, :, wo0:wo1], in0=x1r[:, :, wo0+dx-1:wo1+dx-1],
                            scalar=sc, in1=ar[:, :, wo0:wo1], op0=AL.mult, op1=AL.add)
                    elif dx == 1:
                        # pure y-shift: contiguous 240-element run per t-block
                        ho0 = 1 if dy == 0 else 0
                        o0 = ho0 * 16
                        i0 = o0 + (dy - 1) * 16
                        VE.scalar_tensor_tensor(
                            out=at4[:, :, o0:o0+240], in0=x1t[:, :, i0:i0+240],
                            scalar=sc, in1=at4[:, :, o0:o0+240], op0=AL.mult, op1=AL.add)
                    else:
                        # corner: full run per t-block + negated-tap edge fix
                        ho0 = 1 if dy == 0 else 0
                        o0 = ho0 * 16
                        delta = 16 * (dy - 1) + (dx - 1)
                        s_ = o0 + max(0, -(o0 + delta))
                        e_ = o0 + 240 - max(0, (o0 + delta + 240) - 256)
                        VE.scalar_tensor_tensor(
                            out=at4[:, :, s_:e_], in0=x1t[:, :, s_+delta:e_+delta],
                            scalar=sc, in1=at4[:, :, s_:e_], op0=AL.mult, op1=AL.add)
                        we = 0 if dx == 0 else 15
                        pos = [h_*16+we for h_ in range(ho0, ho0+15)
                               if s_ <= h_*16+we < e_]
                        p0, np_ = pos[0], len(pos)
                        hs, ws = (p0 + delta) // 16, (p0 + delta) % 16
                        of4 = acc.rearrange("p (t h w) -> p t h w", t=4, h=16)
                        if4 = x1f.rearrange("p (t h w) -> p t h w", t=4, h=16)
                        ofx = of4[:, :, p0//16:p0//16+np_, we:we+1].rearrange(
                            "p t h w -> p t (h w)")
                        ifx = if4[:, :, hs:hs+np_, ws:ws+1].rearrange(
                            "p t h w -> p t (h w)")
                        VE.scalar_tensor_tensor(
                            out=ofx, in0=ifx, scalar=dwtn[:, ct:ct+1],
                            in1=ofx, op0=AL.mult, op1=AL.add)
            # dw-LIF + gate -> mg (bf16)
            m_dw = mdw_p.tile([P, N], F32, name=f"m_dw{ch}", tag="t")
            mg = mg_p.tile([P, 1024], BF16, name=f"mg{ch}", tag="t")
            for t in range(T):
                thr = float(2.0 ** t)
                U = acc[:, t*N:(t+1)*N]
                if t > 0:
                    VE.scalar_tensor_tensor(out=U, in0=U, scalar=thr, in1=m_dw,
                                            op0=AL.mult, op1=AL.add)
                VE.scalar_tensor_tensor(out=mg[:, t*N:(t+1)*N], in0=U, scalar=thr,
                                        in1=sp_ch[:, 1024 + t*N: 1024 + (t+1)*N],
                                        op0=AL.is_ge, op1=AL.mult)
                if t < T - 1:
                    VE.scalar_tensor_tensor(out=m_dw, in0=U, scalar=thr, in1=U,
                                            op0=AL.is_lt, op1=AL.mult)
            # fc2 accumulate (2-term bf16, rhs exact)
            for tp in range(2):
                for mt in range(3):
                    nc.tensor.matmul(po[tp][mt], f2h[ch][:, mt*P:(mt+1)*P],
                                     mg[:, tp*512:(tp+1)*512],
                                     start=(ch == 0), stop=False, skip_group_check=True)
                    nc.tensor.matmul(po[tp][mt], f2l[ch][:, mt*P:(mt+1)*P],
                                     mg[:, tp*512:(tp+1)*512],
                                     start=False, stop=(ch == 7), skip_group_check=True)

        # fc2 evict + final LIF + residual + store
        xo_t = [xev_p.tile([P, 768], F32, name=f"xo{t}", tag="t") for t in range(T)]
        for t in range(T):
            for mt in range(3):
                c = mt * 4 + t
                nc.scalar.activation(xo_t[t][:, mt*N:(mt+1)*N],
                                     po[t // 2][mt][:, (t % 2)*N:(t % 2+1)*N],
                                     AF.Identity, bias=b_o[:, c:c+1], scale=a_o[:, c:c+1])
        for t in range(T):
            thr = float(2.0 ** t)
            U = xo_t[t]
            if t > 0:
                GE.tensor_add(U, m_o, U)
            if t < T - 1:
                VE.scalar_tensor_tensor(out=m_o, in0=U, scalar=thr, in1=U,
                                        op0=AL.is_lt, op1=AL.mult)
            for mt in range(3):
                # final out in place over xo (reset already consumed U)
                VE.scalar_tensor_tensor(
                    out=U[:, mt*N:(mt+1)*N], in0=U[:, mt*N:(mt+1)*N], scalar=thr,
                    in1=xs_kt[mt][:, t*N:(t+1)*N], op0=AL.is_ge, op1=AL.add)
                nc.sync.dma_start(out=d['out'][t*C + mt*P: t*C + (mt+1)*P, :],
                                  in_=U[:, mt*N:(mt+1)*N])


def _build():
    nc = bacc.Bacc()
    with tile.TileContext(nc) as tc:
        with tc.tile_pool(name="dram", bufs=1, space="DRAM") as dram:
            def din(name, shape, dt=F32):
                return dram.tile(shape, dt, kind="ExternalInput", name=name,
                                 uniquify=False)
            d = {
                'xin': din('xin', [T * C, N]),
                'out': dram.tile([T * C, N], F32, kind="ExternalOutput",
                                 name='out', uniquify=False),
                'kq_whi': din('kq_whi', [384, 480], BF16),
                'kq_wlo': din('kq_wlo', [384, 480], BF16),
                'a_kq': din('a_kq', [96, 20]),
                'b_kq': din('b_kq', [96, 20]),
                'v_whi': din('v_whi', [384, 384], BF16),
                'v_wlo': din('v_wlo', [384, 384], BF16),
                'r_wT': din('r_wT', [384, 4]),
                'r_b': din('r_b', [1, 4]),
                'ones': din('ones', [1, 128]),
                'pj_whi': din('pj_whi', [384, 384], BF16),
                'pj_wlo': din('pj_wlo', [384, 384], BF16),
                'a_p': din('a_p', [128, 12]),
                'b_p': din('b_p', [128, 12]),
                'f1_whi': din('f1_whi', [384, 2048], BF16),
                'f1_wlo': din('f1_wlo', [384, 2048], BF16),
                'a_h': din('a_h', [128, 64]),
                'b_h': din('b_h', [128, 64]),
                'dw_tap': din('dw_tap', [128, 72]),
                'dw_tapn': din('dw_tapn', [128, 72]),
                'b_dw': din('b_dw', [128, 8]),
                'f2_whi': din('f2_whi', [1024, 384], BF16),
                'f2_wlo': din('f2_wlo', [1024, 384], BF16),
                'a_o': din('a_o', [128, 12]),
                'b_o': din('b_o', [128, 12]),
                'ident': din('ident', [128, 128], BF16),
            }
            _body(nc, tc, d)
    nc.finalize()
    return nc


_NC_CACHE = {}


def _get_nc():
    if 'nc' not in _NC_CACHE:
        _NC_CACHE['nc'] = _build()
    return _NC_CACHE['nc']


def _tcols(a):
    rows, k = a.shape
    out = np.empty((rows, k * 4), np.float32)
    for u in range(k):
        for t in range(4):
            out[:, u * 4 + t] = a[:, u] * (2.0 ** t)
    return out


def _split(w):
    hi = w.astype(ml_dtypes.bfloat16)
    lo = (w - hi.astype(np.float32)).astype(ml_dtypes.bfloat16)
    return hi, lo


def _prep_common(inputs):
    inp = {k: np.asarray(v, np.float32) for k, v in inputs.items()}
    k_wT = inp['k_w'].T
    exp_wT = np.concatenate([inp['exp_w'][e].T for e in range(NE)], axis=1)
    kq_wT = np.concatenate([k_wT, exp_wT], axis=1)
    a_kq = np.zeros((96, 5), np.float32)
    b_kq = np.zeros((96, 5), np.float32)
    a_kq[:, 0] = 0.5
    for e in range(NE):
        a_kq[:, 1 + e] = 0.5 * inp['exp_g'][e] * S
        b_kq[:, 1 + e] = 0.5 * inp['exp_b'][e]
    taps = inp['dw_w'][:, 0] * (0.5 * inp['dw_g'] * S)[:, None, None]
    kqh, kqlo = _split(kq_wT)
    vhh, vlo = _split(inp['v_w'].T)
    pjh_, pjlo = _split(inp['proj_w'].T)
    f1h_, f1lo = _split(inp['fc1_w'].T)
    f2h_, f2lo = _split(inp['fc2_w'].T)
    com = {
        'kq_whi': kqh, 'kq_wlo': kqlo,
        'a_kq': _tcols(a_kq), 'b_kq': _tcols(b_kq),
        'v_whi': vhh, 'v_wlo': vlo,
        'r_wT': inp['router_w'].T * (inp['router_g'] * S * 0.5)[None, :],
        'r_b': (0.5 * (inp['router_b'] * inp['router_g'] * S
                       + inp['router_be'])).reshape(1, 4),
        'ones': np.ones((1, 128), np.float32),
        'pj_whi': pjh_, 'pj_wlo': pjlo,
        'a_p': _tcols((0.5 * inp['proj_g'] * S).reshape(3, 128).T),
        'b_p': _tcols((0.5 * (inp['proj_b'] * inp['proj_g'] * S
                              + inp['proj_be'])).reshape(3, 128).T),
        'f1_whi': f1h_, 'f1_wlo': f1lo,
        'a_h': _tcols((0.5 * inp['fc1_g'] * S).reshape(16, 128).T),
        'b_h': _tcols((0.5 * (inp['fc1_b'] * inp['fc1_g'] * S
                              + inp['fc1_be'])).reshape(16, 128).T),
        'dw_tap': taps.reshape(8, 128, 9).transpose(1, 0, 2).reshape(128, 72),
        'dw_tapn': -taps.reshape(8, 128, 9).transpose(1, 0, 2).reshape(128, 72),
        'b_dw': (0.5 * (inp['dw_b'] * inp['dw_g'] * S
                        + inp['dw_be'])).reshape(8, 128).T,
        'f2_whi': f2h_, 'f2_wlo': f2lo,
        'a_o': _tcols((0.5 * inp['fc2_g'] * S).reshape(3, 128).T),
        'b_o': _tcols((0.5 * (inp['fc2_b'] * inp['fc2_g'] * S
                              + inp['fc2_be'])).reshape(3, 128).T),
        'ident': np.eye(128, dtype=ml_dtypes.bfloat16),
    }
    return {k: np.ascontiguousarray(v) for k, v in com.items()}


def run(inputs, trace=False, tmpdir=None):
    com = _prep_common(inputs)
    x = np.asarray(inputs['x'], np.float32).reshape(T, B, C, N)
    in_maps = []
    for b in range(B):
        m = dict(com)
        m['xin'] = np.ascontiguousarray(x[:, b].reshape(T * C, N))
        in_maps.append(m)
    res = run_bass_kernel_spmd(_get_nc(), in_maps, list(range(B)),
                               trace=trace, tmpdir=tmpdir)
    out = np.empty((T, B, C, N), np.float32)
    for b in range(B):
        out[:, b] = res.results[b]['out'].reshape(T, C, N)
    return out.reshape(T * B, C, 16, 16), res.exec_time_ns


def kernel(**inputs):
    out, _ = run(inputs)
    return out
